# revision 28
# baseline (speedup 1.0000x reference)
"""Bidirectional Mamba block — Trainium2 Bass/Tile kernel, 8-core data-parallel.

Sharding: batch B=8 -> one sample per NeuronCore, zero collectives.

Per-core layout: activations transposed ([channel, time]).  The depthwise
conv (k=2) is folded into the in-projection matmul (W0 = cw0*in_w streamed
against x shifted by one step, W1 = cw1*in_w against x), so conv+silu is a
PSUM accumulation followed by one ACT Silu — P1 needs no vector-engine work
at all.  The selective scan runs as ONE hardware `tensor_tensor_scan` per
128-channel tile covering all 16 states: per-state segments of length TC+2
are concatenated with two reset columns (a=0,0 / b=0,carry — even segment
starts keep fp16 2x-mode alignment) so state never leaks across segments.
The scan's internal state is fp32; fp16 is only the observed h.  y=sum_s C*h
is a fp16 2x tensor_tensor multiply (aliased into the dead b-stream buffer)
plus four in-place fp16 tree folds.  dt softplus is Exp+Ln (both live in the
natural_log_exp ACT table set, like the scan exps).  u/B/C/h/y/g and the
out-projection weights are fp16 (DVE 2x, scan recurrence unaffected);
in/x/xproj/dt GEMM paths stay f32r.  P1 of chunk c+1 overlaps P2 of chunk c
(per-direction DRAM scratch for xc/silu(z)); both directions share one pool
scope so dir-2's P1 hides under dir-1's scan tail.  Phase C (LN+FFN+LN)
caches c1/c2 weights in SBUF freed by the scan pools, all fp16.

Measured (clean clock): ~2.43 ms vs 3.96 ms baseline; DVE-bound at ~87%
occupancy, of which the scan op itself (fixed ~2.1 ns/elem HW rate) is half.
"""

import numpy as np

import concourse.bass as bass
import concourse.bacc as bacc
import concourse.mybir as mybir
from concourse import tile
from concourse import bass_utils

AL = mybir.AluOpType
AF = mybir.ActivationFunctionType
F32 = mybir.dt.float32
F32R = mybir.dt.float32r
F16 = mybir.dt.float16

NCORES = 8
MMF = 512
MMDT = F32R                    # dtype for main matmul-feeding tensors
SDT = F16                      # scan-adjacent dtype (a, b, h, hC, B/C bcast)


class Dims:
    def __init__(self, L=1024, D=512, DI=2048, DS=16, DTR=32, DFF=2048, TC=512):
        self.L, self.D, self.DI, self.DS, self.DTR, self.DFF = L, D, DI, DS, DTR, DFF
        self.TC = TC
        self.NTC = L // TC
        self.KD = D // 128
        self.KI = DI // 128
        self.KF = DFF // 128
        self.SEG = TC + 2                      # per-state scan segment: [0,carry | data]
        assert L % TC == 0 and DS == 16 and DTR == 32


DIMS = Dims()


def r32(ap):
    return ap.bitcast(F32R)


# -------------------------------------------------------------------- builder
def build_program(dm: Dims = DIMS, loop_n=0, sim_silu=False):
    nc = bacc.Bacc("TRN2", target_bir_lowering=False, debug=False)

    L, D, DI, DS, DTR = dm.L, dm.D, dm.DI, dm.DS, dm.DTR
    dram = {}

    def din(name, shape, dt=F32):
        dram[name] = nc.dram_tensor(name, list(shape), dt,
                                    kind="ExternalInput").ap()

    # x with a leading zero column (for the conv's one-step shift)
    din("xTp", (D, L + 1), MMDT); din("xTrp", (D, L + 1), MMDT)
    din("ones", (128, 128), MMDT)
    din("sel", (DTR, 2 * DS * 128), F16)
    for p in ("m1_", "m2_"):
        din(p + "W01T", (D, 2 * DI), MMDT)          # [W0 | W1] conv-folded
        din(p + "in_wzT", (D, DI), MMDT)            # z half of in_w
        din(p + "xproj_wT", (DI, DTR + 2 * DS), F16)
        din(p + "dt_wT", (DTR, DI), MMDT)
        din(p + "out_wT", (DI, D), F16)
        din(p + "A", (DI, DS))                      # -exp(A_log)
        din(p + "dt_b", (128, dm.KI))
        din(p + "cb", (128, dm.KI))                 # conv_b
        din(p + "Dp", (128, dm.KI))
    din("ln_g", (128, dm.KD)); din("ln_b", (128, dm.KD))
    din("c1_wT", (D, dm.DFF), MMDT); din("c1_b", (128, dm.KF))
    din("c2_wT", (dm.DFF, D), MMDT); din("c2_b", (128, dm.KD))
    outT = nc.dram_tensor("outT", [D, L], F32, kind="ExternalOutput").ap()

    with tile.TileContext(nc) as tc_:
        if loop_n:
            with tc_.For_i(0, loop_n, 1):
                _emit(nc, tc_, dram, outT, dm, sim_silu)
        else:
            _emit(nc, tc_, dram, outT, dm, sim_silu)
    nc.compile()
    return nc


def _emit(nc, tc_, dram, outT, dm, sim_silu=False):
    from contextlib import ExitStack
    L, D, DI, DS, DTR, DFF, TC, NTC = (dm.L, dm.D, dm.DI, dm.DS, dm.DTR,
                                       dm.DFF, dm.TC, dm.NTC)
    KD, KI, KF, SEG = dm.KD, dm.KI, dm.KF, dm.SEG
    NST = DS * SEG                               # scan stream length per tile
    mm = nc.tensor.matmul

    with ExitStack() as ctx:
        pers = ctx.enter_context(tc_.tile_pool(name="pers", bufs=1))
        wp = ctx.enter_context(tc_.tile_pool(name="wp", bufs=2))
        psmm = ctx.enter_context(tc_.tile_pool(name="psmm", bufs=2, space="PSUM"))
        psacc = ctx.enter_context(tc_.tile_pool(name="psacc", bufs=1, space="PSUM"))
        dpool = ctx.enter_context(tc_.tile_pool(name="dpool", bufs=1, space="DRAM"))

        ones_sb = pers.tile([128, 128], MMDT, tag="ones", name="ones")
        nc.sync.dma_start(ones_sb[:], dram["ones"][:])
        ones16 = pers.tile([128, 128], F16, tag="ones16", name="ones16")
        nc.gpsimd.dma_start(ones16[:], dram["ones"][:])
        eps_sb = pers.tile([128, 1], F32, tag="eps", name="eps")
        nc.vector.memset(eps_sb[:], 1e-5)

        y_scr = [dpool.tile([D, L], F32, tag=f"y_scr{i}", name=f"y_scr{i}")
                 for i in range(2)]
        xc_scrs = [dpool.tile([DI, L], F16, tag=f"xc_scr{i}", name=f"xc_scr{i}")
                   for i in range(2)]
        sz_scrs = [dpool.tile([DI, L], F16, tag=f"sz_scr{i}", name=f"sz_scr{i}")
                   for i in range(2)]

        # ====================================================== SSM directions
        with tc_.tile_pool(name="dirp", bufs=1) as dirp, \
             tc_.tile_pool(name="dsp", bufs=2) as sp, \
             tc_.tile_pool(name="dsp1", bufs=1) as sp1:
          sel_sb = dirp.tile([DTR, 2 * DS * 128], F16, tag="sel", name="sel")
          nc.sync.dma_start(sel_sb[:], dram["sel"][:])
          for di_ in range(2):
            p = ("m1_", "m2_")[di_]
            xnm = ("xTp", "xTrp")[di_]
            xc_scr, sz_scr = xc_scrs[di_], sz_scrs[di_]
            A_sb = dirp.tile([128, KI * DS], F32, tag=p + "A", name="A")
            nc.sync.dma_start(
                A_sb[:].rearrange("q (k s) -> q k s", k=KI),
                dram[p + "A"].rearrange("(k q) s -> q k s", q=128))
            vec = {}
            for nm in ("dt_b", "cb", "Dp"):
                vec[nm] = dirp.tile([128, KI], F32, tag=p + nm, name=nm)
                nc.sync.dma_start(vec[nm][:], dram[p + nm][:])
            xpw_sb = dirp.tile([128, KI * (DTR + 2 * DS)], F16, tag=p + "xpw",
                               name="xpw")
            nc.sync.dma_start(
                xpw_sb[:].rearrange("q (k c) -> q k c", k=KI),
                dram[p + "xproj_wT"].rearrange("(k q) c -> q k c", q=128))
            dtw_sb = dirp.tile([DTR, DI], MMDT, tag=p + "dtw", name="dtw")
            nc.sync.dma_start(dtw_sb[:], dram[p + "dt_wT"][:])
            carry = dirp.tile([128, KI * DS], F32, tag="carry", name="carry")
            nc.vector.memset(carry[:], 0.0)
            dbc_sb = dirp.tile([64, TC], MMDT, tag="dbc", name="dbc")
            dbc32 = dirp.tile([32, TC], F16, tag="dbc32", name="dbc32")

            for tcix in range(NTC):
                t0 = tcix * TC
                # x window [t0-1, t0+TC) via the zero-padded xTp (col t of
                # xTp = x col t-1)
                xw = [sp.tile([128, TC + 1], MMDT, tag=f"xw{k}", name=f"xw{k}",
                              bufs=1) for k in range(KD)]
                for k in range(KD):
                    nc.sync.dma_start(
                        xw[k][:],
                        dram[xnm][k * 128:(k + 1) * 128, t0:t0 + TC + 1])

                # ---- P1 (Silu table): xc, dbc accumulation, silu(z) --------
                dbc_ps = psacc.tile([64, TC], F32, tag="acc_dbc", name="acc_dbc")
                for kt in range(KI):
                    ps = psmm.tile([128, TC], F32, tag="mm", name="mm")
                    w4 = wp.tile([128, 2 * KD * 128], MMDT, tag="w_in", name="w_in")
                    w01_r = dram[p + "W01T"].rearrange("(k q) e -> q k e", q=128)
                    nc.gpsimd.dma_start(
                        w4[:, 0:KD * 128].rearrange("q (k e) -> q k e", k=KD),
                        w01_r[:, :, kt * 128:(kt + 1) * 128])
                    nc.gpsimd.dma_start(
                        w4[:, KD * 128:].rearrange("q (k e) -> q k e", k=KD),
                        w01_r[:, :, DI + kt * 128:DI + (kt + 1) * 128])
                    for k in range(KD):            # W0 @ x_prev + W1 @ x
                        mm(ps[:], r32(w4[:, k * 128:(k + 1) * 128]),
                           xw[k][:, 0:TC], start=(k == 0), stop=False)
                    for k in range(KD):
                        mm(ps[:], r32(w4[:, (KD + k) * 128:(KD + k + 1) * 128]),
                           xw[k][:, 1:TC + 1], start=False, stop=(k == KD - 1))
                    xck = sp.tile([128, TC], F16, tag="xck", name="xck", bufs=1)
                    if sim_silu:
                        sg_ = sp.tile([128, TC], F32, tag="sg_", name="sg_", bufs=1)
                        vv_ = sp.tile([128, TC], F32, tag="vv_", name="vv_", bufs=1)
                        nc.scalar.activation(sg_[:], ps[:], AF.Sigmoid,
                                             bias=vec["cb"][:, kt:kt + 1])
                        nc.vector.tensor_scalar(vv_[:], ps[:],
                                                vec["cb"][:, kt:kt + 1],
                                                None, AL.add)
                        nc.vector.tensor_tensor(xck[:], sg_[:], vv_[:], AL.mult)
                    else:
                        nc.scalar.activation(xck[:], ps[:], AF.Silu,
                                             bias=vec["cb"][:, kt:kt + 1])
                    nc.gpsimd.dma_start(xc_scr[kt * 128:(kt + 1) * 128, t0:t0 + TC], xck[:])
                    mm(dbc_ps[:], xpw_sb[:, kt * 64:(kt + 1) * 64], xck[:],
                       start=(kt == 0), stop=(kt == KI - 1))

                    zps = psmm.tile([128, TC], F32, tag="mm", name="mm")
                    wz = wp.tile([128, KD * 128], MMDT, tag="w_z", name="w_z")
                    nc.gpsimd.dma_start(
                        wz[:].rearrange("q (k e) -> q k e", k=KD),
                        dram[p + "in_wzT"].rearrange("(k q) e -> q k e", q=128)
                        [:, :, kt * 128:(kt + 1) * 128])
                    for k in range(KD):
                        mm(zps[:], r32(wz[:, k * 128:(k + 1) * 128]),
                           xw[k][:, 1:TC + 1], start=(k == 0), stop=(k == KD - 1))
                    szk = sp.tile([128, TC], F16, tag="szk", name="szk", bufs=1)
                    if sim_silu:
                        sgz_ = sp.tile([128, TC], F32, tag="sgz_", name="sgz_", bufs=1)
                        nc.scalar.activation(sgz_[:], zps[:], AF.Sigmoid)
                        nc.vector.tensor_tensor(
                            szk[:], sgz_[:], zps[:], AL.mult)
                    else:
                        nc.scalar.activation(szk[:], zps[:], AF.Silu)
                    nc.gpsimd.dma_start(sz_scr[kt * 128:(kt + 1) * 128, t0:t0 + TC], szk[:])
                nc.scalar.copy(dbc_sb[:], dbc_ps[:])
                nc.scalar.copy(dbc32[:], dbc_sb[DTR:DTR + 2 * DS, :])

                # ---- broadcast B_s / C_s via selection matmuls -------------
                bcB = dirp.tile([128, DS * TC], SDT, tag=f"bcB{tcix % 2}",
                                name="bcB")
                bcC = dirp.tile([128, DS * TC], SDT, tag=f"bcC{tcix % 2}",
                                name="bcC")
                for s in range(2 * DS):
                    bps = psmm.tile([128, TC], F32, tag="mm", name="mm")
                    mm(bps[:], sel_sb[:, s * 128:(s + 1) * 128], dbc32[:],
                       start=True, stop=True)
                    dst = (bcB[:, s * TC:(s + 1) * TC] if s < DS
                           else bcC[:, (s - DS) * TC:(s - DS + 1) * TC])
                    nc.scalar.copy(dst, bps[:])

                # ---- P2 (Exp/Ln table): softplus dt, scan, y ---------------
                y_ps = [psacc.tile([128, TC], F32, tag=f"acc{k}", name=f"acc{k}")
                        for k in range(KD)]
                for kt in range(KI):
                    dps = psmm.tile([128, TC], F32, tag="mm", name="mm")
                    mm(dps[:], dtw_sb[:, kt * 128:(kt + 1) * 128],
                       dbc_sb[0:DTR, :], start=True, stop=True)
                    t1 = sp1.tile([128, TC], F32, tag="sp_t1", name="sp_t1")
                    nc.scalar.activation(t1[:], dps[:], AF.Exp,
                                         bias=vec["dt_b"][:, kt:kt + 1])
                    dts = sp.tile([128, TC], SDT, tag="dts", name="dts")
                    nc.scalar.activation(dts[:], t1[:], AF.Ln, bias=1.0)

                    # a-stream: per-state segments [0 | exp(A_s*dt)*TC]
                    a_st = sp.tile([128, NST], SDT, tag="a_st", name="a_st")
                    nc.vector.memset(
                        a_st[:].rearrange("q (s t) -> q s t", s=DS)[:, :, 0:2],
                        0.0)
                    for s in range(DS):
                        nc.scalar.activation(
                            a_st[:, s * SEG + 2:(s + 1) * SEG], dts[:], AF.Exp,
                            scale=A_sb[:, kt * DS + s:kt * DS + s + 1])

                    xck = sp.tile([128, TC], F16, tag="xck2", name="xck2")
                    nc.gpsimd.dma_start(xck[:], xc_scr[kt * 128:(kt + 1) * 128, t0:t0 + TC])
                    u = sp1.tile([128, TC], SDT, tag="u", name="u")
                    nc.vector.tensor_tensor(u[:], xck[:], dts[:], AL.mult)

                    # b-stream: [carry_s | u*B_s]
                    b_st = sp1.tile([128, NST], SDT, tag="b_st", name="b_st")
                    nc.vector.memset(b_st[:, 0::SEG], 0.0)
                    nc.vector.tensor_copy(b_st[:, 1::SEG],
                                          carry[:, kt * DS:(kt + 1) * DS])
                    SG = 4
                    uv = u[:].rearrange("q (o t) -> q o t", o=1)
                    for sg in range(DS // SG):
                        nc.vector.tensor_tensor(
                            b_st[:].rearrange("q (s t) -> q s t", s=DS)
                            [:, sg * SG:(sg + 1) * SG, 2:SEG],
                            uv.to_broadcast((128, SG, TC)),
                            bcB[:, sg * SG * TC:(sg + 1) * SG * TC]
                            .rearrange("q (s t) -> q s t", s=SG), AL.mult)

                    h = sp1.tile([128, NST], SDT, tag="h", name="h")
                    nc.vector.tensor_tensor_scan(h[:], a_st[:], b_st[:], 0.0,
                                                 AL.mult, AL.add)
                    nc.vector.tensor_copy(carry[:, kt * DS:(kt + 1) * DS],
                                          h[:, TC + 1::SEG])

                    # y_t = sum_s C_s(t) h_s(t):  mult (2x fp16) + tree + reduce
                    hC = b_st
                    nc.vector.tensor_tensor(
                        hC[:, 0:DS * TC].rearrange("q (s t) -> q s t", s=DS),
                        h[:].rearrange("q (s t) -> q s t", s=DS)[:, :, 2:SEG],
                        bcC[:].rearrange("q (s t) -> q s t", s=DS), AL.mult)
                    nc.vector.tensor_tensor(hC[:, 0:DS * TC // 2],
                                            hC[:, 0:DS * TC // 2],
                                            hC[:, DS * TC // 2:DS * TC], AL.add)
                    nc.vector.tensor_tensor(hC[:, 0:DS * TC // 4],
                                            hC[:, 0:DS * TC // 4],
                                            hC[:, DS * TC // 4:DS * TC // 2],
                                            AL.add)
                    nc.vector.tensor_tensor(hC[:, 0:DS * TC // 8],
                                            hC[:, 0:DS * TC // 8],
                                            hC[:, DS * TC // 8:DS * TC // 4],
                                            AL.add)
                    yv = sp.tile([128, TC], SDT, tag="yv", name="yv")
                    nc.vector.tensor_tensor(yv[:], hC[:, 0:TC],
                                            hC[:, TC:2 * TC], AL.add)

                    # y += Dp*xc (Dp*xc on ACT: per-partition scale)
                    dpx = sp.tile([128, TC], SDT, tag="dpx", name="dpx")
                    nc.scalar.activation(dpx[:], xck[:], AF.Copy,
                                         scale=vec["Dp"][:, kt:kt + 1])
                    yf = sp.tile([128, TC], SDT, tag="yf", name="yf")
                    nc.vector.tensor_tensor(yf[:], yv[:], dpx[:], AL.add)
                    szk = sp.tile([128, TC], F16, tag="szk2", name="szk2")
                    nc.gpsimd.dma_start(szk[:], sz_scr[kt * 128:(kt + 1) * 128, t0:t0 + TC])
                    g = sp.tile([128, TC], F16, tag="g", name="g", bufs=2)
                    nc.vector.tensor_tensor(g[:], yf[:], szk[:], AL.mult)

                    w4 = wp.tile([128, KD * 128], F16, tag="w_out", name="w_out")
                    nc.gpsimd.dma_start(
                        w4[:], dram[p + "out_wT"][kt * 128:(kt + 1) * 128, :])
                    for k in range(KD):
                        mm(y_ps[k][:], w4[:, k * 128:(k + 1) * 128], g[:],
                           start=(kt == 0), stop=(kt == KI - 1))
                for k in range(KD):
                    yo = sp.tile([128, TC], F32, tag="yo", name="yo", bufs=1)
                    nc.scalar.copy(yo[:], y_ps[k][:])
                    nc.sync.dma_start(
                        y_scr[di_][k * 128:(k + 1) * 128, t0:t0 + TC], yo[:])

        # ============================================================ phase C
        with tc_.tile_pool(name="cpool", bufs=1) as cp, \
             tc_.tile_pool(name="csp", bufs=2) as sp:
            ln_g = cp.tile([128, KD], F32, tag="ln_g", name="ln_g")
            ln_b = cp.tile([128, KD], F32, tag="ln_b", name="ln_b")
            c1b = cp.tile([128, KF], F32, tag="c1b", name="c1b")
            c2b = cp.tile([128, KD], F32, tag="c2b", name="c2b")
            for nm, t in (("ln_g", ln_g), ("ln_b", ln_b), ("c1_b", c1b),
                          ("c2_b", c2b)):
                nc.sync.dma_start(t[:], dram[nm][:])
            c1w_sb = cp.tile([128, KD * DFF], F16, tag="c1w", name="c1w")
            nc.gpsimd.dma_start(
                c1w_sb[:].rearrange("q (k e) -> q k e", k=KD),
                dram["c1_wT"].rearrange("(k q) e -> q k e", q=128))
            c2w_sb = cp.tile([128, KF * D], F16, tag="c2w", name="c2w")
            nc.gpsimd.dma_start(
                c2w_sb[:].rearrange("q (f e) -> q f e", f=KF),
                dram["c2_wT"].rearrange("(f q) e -> q f e", q=128))
            CH = min(MMF, L)

            def layernorm(in_tiles, out_tiles):
                f16in = True
                one_st = ones16
                sums = cp.tile([1, L], MMDT, tag="ln_srow", name="ln_srow")
                sqs = cp.tile([1, L], MMDT, tag="ln_qrow", name="ln_qrow")
                for nk in range(0, L, CH):
                    sps = psacc.tile([1, CH], F32, tag="mmrow", name="mmrow")
                    for k in range(KD):
                        mm(sps[:], one_st[:, 0:1],
                           in_tiles[k][:, nk:nk + CH],
                           start=(k == 0), stop=(k == KD - 1))
                    nc.scalar.copy(sums[:, nk:nk + CH], sps[:])
                for nk in range(0, L, CH):
                    qps = psacc.tile([1, CH], F32, tag="mmrow", name="mmrow")
                    for k in range(KD):
                        sq = sp.tile([128, CH], F16, tag="ln_sq", name="ln_sq")
                        nc.vector.tensor_tensor(sq[:], in_tiles[k][:, nk:nk + CH],
                                                in_tiles[k][:, nk:nk + CH], AL.mult)
                        mm(qps[:], one_st[:, 0:1], sq[:],
                           start=(k == 0), stop=(k == KD - 1))
                    nc.scalar.copy(sqs[:, nk:nk + CH], qps[:])
                mu = cp.tile([128, L], F32, tag="ln_mu", name="ln_mu")
                inv = cp.tile([128, L], F32, tag="ln_inv", name="ln_inv")
                for nk in range(0, L, CH):
                    mps = psmm.tile([128, CH], F32, tag="mm", name="mm")
                    mm(mps[:], r32(ones_sb[0:1, :]), sums[:, nk:nk + CH],
                       start=True, stop=True)
                    nc.vector.tensor_scalar(mu[:, nk:nk + CH], mps[:], 1.0 / D,
                                            None, AL.mult)
                    qrep = psmm.tile([128, CH], F32, tag="mm", name="mm")
                    mm(qrep[:], r32(ones_sb[0:1, :]), sqs[:, nk:nk + CH],
                       start=True, stop=True)
                    ex2 = sp.tile([128, CH], F32, tag="ln_ex2", name="ln_ex2")
                    nc.vector.tensor_scalar(ex2[:], qrep[:], 1.0 / D, None, AL.mult)
                    var = sp.tile([128, CH], F32, tag="ln_var", name="ln_var")
                    nc.vector.tensor_tensor(var[:], mu[:, nk:nk + CH],
                                            mu[:, nk:nk + CH], AL.mult)
                    nc.vector.tensor_tensor(var[:], ex2[:], var[:], AL.subtract)
                    sd = sp.tile([128, CH], F32, tag="ln_sd", name="ln_sd")
                    nc.scalar.activation(sd[:], var[:], AF.Sqrt, bias=eps_sb[:])
                    nc.vector.reciprocal(inv[:, nk:nk + CH], sd[:])
                for k in range(KD):
                    xm = sp.tile([128, L], F32, tag="ln_xm", name="ln_xm")
                    nc.vector.tensor_tensor(xm[:], in_tiles[k][:], mu[:],
                                            AL.subtract)
                    nc.vector.tensor_tensor(xm[:], xm[:], inv[:], AL.mult)
                    nc.vector.tensor_scalar(out_tiles[k][:], xm[:],
                                            ln_g[:, k:k + 1], ln_b[:, k:k + 1],
                                            AL.mult, AL.add)

            y3p = [cp.tile([128, L], F16, tag=f"y3p{k}", name=f"y3p{k}")
                   for k in range(KD)]
            for k in range(KD):
                xt = sp.tile([128, L], MMDT, tag="c_x", name="c_x")
                y1t = sp.tile([128, L], F32, tag="c_y1", name="c_y1")
                y2t = sp.tile([128, L], F32, tag="c_y2", name="c_y2")
                nc.sync.dma_start(xt[:], dram["xTp"][k * 128:(k + 1) * 128, 1:L + 1])
                nc.sync.dma_start(y1t[:], y_scr[0][k * 128:(k + 1) * 128, :])
                nc.sync.dma_start(y2t[:], y_scr[1][k * 128:(k + 1) * 128, :])
                nc.vector.tensor_tensor(y3p[k][:], xt[:], y1t[:], AL.add)
                nc.vector.tensor_tensor(y3p[k][:], y3p[k][:], y2t[:, ::-1], AL.add)
            y3 = [cp.tile([128, L], F16, tag=f"y3_{k}", name=f"y3_{k}")
                  for k in range(KD)]
            layernorm(y3p, y3)

            ypre = y3p
            NFH = min(8, KF)
            for nk in range(0, L, CH):
                yacc = [psacc.tile([128, CH], F32, tag=f"acc{k}", name=f"acc{k}")
                        for k in range(KD)]
                for fh in range(KF // NFH):
                    hbuf = []
                    for f2 in range(NFH):
                        f = fh * NFH + f2
                        hps = psmm.tile([128, CH], F32, tag="mm", name="mm")
                        for k in range(KD):
                            mm(hps[:],
                               c1w_sb[:, k * DFF + f * 128:
                                      k * DFF + (f + 1) * 128],
                               y3[k][:, nk:nk + CH],
                               start=(k == 0), stop=(k == KD - 1))
                        hb = sp.tile([128, CH], F16, tag=f"hb{f2}",
                                     name=f"hb{f2}", bufs=1)
                        nc.scalar.activation(hb[:], hps[:], AF.Relu,
                                             bias=c1b[:, f:f + 1])
                        hbuf.append(hb)
                    for f2 in range(NFH):
                        f = fh * NFH + f2
                        for k in range(KD):
                            mm(yacc[k][:],
                               c2w_sb[:, f * D + k * 128:
                                      f * D + (k + 1) * 128],
                               hbuf[f2][:],
                               start=(f == 0), stop=(f == KF - 1))
                for k in range(KD):
                    nc.vector.scalar_tensor_tensor(
                        ypre[k][:, nk:nk + CH], yacc[k][:], c2b[:, k:k + 1],
                        y3[k][:, nk:nk + CH], AL.add, AL.add)
            outs = [cp.tile([128, L], F32, tag=f"yo_{k}", name=f"yo_{k}")
                    for k in range(KD)]
            layernorm(ypre, outs)
            for k in range(KD):
                nc.sync.dma_start(outT[k * 128:(k + 1) * 128, :],
                                  outs[k][:].bitcast(F32))


# ------------------------------------------------------------------ host side
_PROG_CACHE = {}


def _get_prog():
    if "full" not in _PROG_CACHE:
        _PROG_CACHE["full"] = build_program(DIMS)
    return _PROG_CACHE["full"]


def host_prep(inputs, dm: Dims = DIMS):
    f = np.float32
    x = np.asarray(inputs["x"], dtype=f)
    KI, KD, KF = dm.KI, dm.KD, dm.KF
    DI = dm.DI

    def vt(v, n):
        return np.ascontiguousarray(np.asarray(v, f).reshape(n, 128).T)

    c = {}
    sel = np.zeros((dm.DTR, 2 * dm.DS * 128), f)
    for s in range(2 * dm.DS):
        sel[s, s * 128:(s + 1) * 128] = 1.0
    c["sel"] = sel.astype(np.float16)
    c["ones"] = np.ones((128, 128), f)
    for p in ("m1_", "m2_"):
        in_w = np.asarray(inputs[p + "in_w"], f)          # (2DI, D)
        cw = np.asarray(inputs[p + "conv_w"], f)          # (DI, 2)
        W0 = cw[:, 0:1] * in_w[:DI]                       # (DI, D)
        W1 = cw[:, 1:2] * in_w[:DI]
        c[p + "W01T"] = np.ascontiguousarray(
            np.concatenate([W0, W1], axis=0).T)           # (D, 2DI)
        c[p + "in_wzT"] = np.ascontiguousarray(in_w[DI:].T)
        c[p + "xproj_wT"] = np.ascontiguousarray(
            np.asarray(inputs[p + "xproj_w"], np.float16).T)
        c[p + "dt_wT"] = np.ascontiguousarray(np.asarray(inputs[p + "dt_w"], f).T)
        c[p + "out_wT"] = np.ascontiguousarray(
            np.asarray(inputs[p + "out_w"], np.float16).T)
        c[p + "A"] = np.ascontiguousarray(-np.exp(np.asarray(inputs[p + "A_log"], f)))
        c[p + "dt_b"] = vt(inputs[p + "dt_b"], KI)
        c[p + "cb"] = vt(inputs[p + "conv_b"], KI)
        c[p + "Dp"] = vt(inputs[p + "Dp"], KI)
    c["ln_g"] = vt(inputs["ln_g"], KD)
    c["ln_b"] = vt(inputs["ln_b"], KD)
    c["c1_wT"] = np.ascontiguousarray(np.asarray(inputs["c1_w"], f).T)
    c["c1_b"] = vt(inputs["c1_b"], KF)
    c["c2_wT"] = np.ascontiguousarray(np.asarray(inputs["c2_w"], f).T)
    c["c2_b"] = vt(inputs["c2_b"], KD)

    in_maps = []
    for b in range(x.shape[0]):
        m = dict(c)
        xp = np.zeros((dm.D, dm.L + 1), f)
        xp[:, 1:] = x[b].T
        m["xTp"] = xp
        xrp = np.zeros((dm.D, dm.L + 1), f)
        xrp[:, 1:] = x[b][::-1].T
        m["xTrp"] = xrp
        in_maps.append(m)
    return in_maps


def kernel(**inputs):
    nc = _get_prog()
    in_maps = host_prep(inputs)
    res = bass_utils.run_bass_kernel_spmd(nc, in_maps, core_ids=list(range(NCORES)))
    return np.stack([np.ascontiguousarray(o["outT"].T) for o in res.results], axis=0)


# revision 30
# speedup vs baseline: 1.0162x; 1.0162x over previous
"""Bidirectional Mamba block — Trainium2 Bass/Tile kernel, 8-core data-parallel.

Sharding: batch B=8 -> one sample per NeuronCore, zero collectives.

Per-core layout: activations transposed ([channel, time]).  The depthwise
conv (k=2) is folded into the in-projection matmul (W0 = cw0*in_w streamed
against x shifted by one step, W1 = cw1*in_w against x), so conv+silu is a
PSUM accumulation followed by one ACT Silu — P1 needs no vector-engine work
at all.  The selective scan runs as ONE hardware `tensor_tensor_scan` per
128-channel tile covering all 16 states: per-state segments of length TC+2
are concatenated with two reset columns (a=0,0 / b=0,carry — even segment
starts keep fp16 2x-mode alignment) so state never leaks across segments.
The scan's internal state is fp32; fp16 is only the observed h.  y=sum_s C*h
is a fp16 2x tensor_tensor multiply (aliased into the dead b-stream buffer)
plus four in-place fp16 tree folds.  dt softplus is Exp+Ln (both live in the
natural_log_exp ACT table set, like the scan exps).  u/B/C/h/y/g and the
out-projection weights are fp16 (DVE 2x, scan recurrence unaffected);
in/x/xproj/dt GEMM paths stay f32r.  P1 of chunk c+1 overlaps P2 of chunk c
(per-direction DRAM scratch for xc/silu(z)); both directions share one pool
scope so dir-2's P1 hides under dir-1's scan tail.  Phase C (LN+FFN+LN)
caches c1/c2 weights in SBUF freed by the scan pools, all fp16.

Measured (clean clock): ~2.43 ms vs 3.96 ms baseline; DVE-bound at ~87%
occupancy, of which the scan op itself (fixed ~2.1 ns/elem HW rate) is half.
"""

import numpy as np

import concourse.bass as bass
import concourse.bacc as bacc
import concourse.mybir as mybir
from concourse import tile
from concourse import bass_utils

AL = mybir.AluOpType
AF = mybir.ActivationFunctionType
F32 = mybir.dt.float32
F32R = mybir.dt.float32r
F16 = mybir.dt.float16

NCORES = 8
MMF = 512
MMDT = F32R                    # dtype for main matmul-feeding tensors
SDT = F16                      # scan-adjacent dtype (a, b, h, hC, B/C bcast)


class Dims:
    def __init__(self, L=1024, D=512, DI=2048, DS=16, DTR=32, DFF=2048, TC=512):
        self.L, self.D, self.DI, self.DS, self.DTR, self.DFF = L, D, DI, DS, DTR, DFF
        self.TC = TC
        self.NTC = L // TC
        self.KD = D // 128
        self.KI = DI // 128
        self.KF = DFF // 128
        self.SEG = TC + 2                      # per-state scan segment: [0,carry | data]
        assert L % TC == 0 and DS == 16 and DTR == 32


DIMS = Dims()


def r32(ap):
    return ap.bitcast(F32R)


# -------------------------------------------------------------------- builder
def build_program(dm: Dims = DIMS, loop_n=0, sim_silu=False):
    nc = bacc.Bacc("TRN2", target_bir_lowering=False, debug=False)

    L, D, DI, DS, DTR = dm.L, dm.D, dm.DI, dm.DS, dm.DTR
    dram = {}

    def din(name, shape, dt=F32):
        dram[name] = nc.dram_tensor(name, list(shape), dt,
                                    kind="ExternalInput").ap()

    # x with a leading zero column (for the conv's one-step shift)
    din("xTp", (D, L + 1), MMDT); din("xTrp", (D, L + 1), MMDT)
    din("ones", (128, 128), MMDT)
    din("sel", (DTR, 2 * DS * 128), F16)
    for p in ("m1_", "m2_"):
        din(p + "W01T", (D, 2 * DI), MMDT)          # [W0 | W1] conv-folded
        din(p + "in_wzT", (D, DI), MMDT)            # z half of in_w
        din(p + "xproj_wT", (DI, DTR + 2 * DS), F16)
        din(p + "dt_wT", (DTR, DI), MMDT)
        din(p + "out_wT", (DI, D), F16)
        din(p + "A", (DI, DS))                      # -exp(A_log)
        din(p + "dt_b", (128, dm.KI))
        din(p + "cb", (128, dm.KI))                 # conv_b
        din(p + "Dp", (128, dm.KI))
    din("ln_g", (128, dm.KD)); din("ln_b", (128, dm.KD))
    din("c1_wT", (D, dm.DFF), MMDT); din("c1_b", (128, dm.KF))
    din("c2_wT", (dm.DFF, D), MMDT); din("c2_b", (128, dm.KD))
    outT = nc.dram_tensor("outT", [D, L], F32, kind="ExternalOutput").ap()

    with tile.TileContext(nc) as tc_:
        if loop_n:
            with tc_.For_i(0, loop_n, 1):
                _emit(nc, tc_, dram, outT, dm, sim_silu)
        else:
            _emit(nc, tc_, dram, outT, dm, sim_silu)
    nc.compile()
    return nc


def _emit(nc, tc_, dram, outT, dm, sim_silu=False):
    from contextlib import ExitStack
    L, D, DI, DS, DTR, DFF, TC, NTC = (dm.L, dm.D, dm.DI, dm.DS, dm.DTR,
                                       dm.DFF, dm.TC, dm.NTC)
    KD, KI, KF, SEG = dm.KD, dm.KI, dm.KF, dm.SEG
    NST = DS * SEG                               # scan stream length per tile
    mm = nc.tensor.matmul

    with ExitStack() as ctx:
        pers = ctx.enter_context(tc_.tile_pool(name="pers", bufs=1))
        wp = ctx.enter_context(tc_.tile_pool(name="wp", bufs=2))
        psmm = ctx.enter_context(tc_.tile_pool(name="psmm", bufs=2, space="PSUM"))
        psacc = ctx.enter_context(tc_.tile_pool(name="psacc", bufs=1, space="PSUM"))
        dpool = ctx.enter_context(tc_.tile_pool(name="dpool", bufs=1, space="DRAM"))

        ones_sb = pers.tile([128, 128], MMDT, tag="ones", name="ones")
        nc.sync.dma_start(ones_sb[:], dram["ones"][:])
        ones16 = pers.tile([128, 128], F16, tag="ones16", name="ones16")
        nc.gpsimd.dma_start(ones16[:], dram["ones"][:])
        eps_sb = pers.tile([128, 1], F32, tag="eps", name="eps")
        nc.vector.memset(eps_sb[:], 1e-5)

        y_scr = [dpool.tile([D, L], F32, tag=f"y_scr{i}", name=f"y_scr{i}")
                 for i in range(2)]
        xc_scrs = [dpool.tile([DI, L], F16, tag=f"xc_scr{i}", name=f"xc_scr{i}")
                   for i in range(2)]
        sz_scrs = [dpool.tile([DI, L], F16, tag=f"sz_scr{i}", name=f"sz_scr{i}")
                   for i in range(2)]

        # ====================================================== SSM directions
        with tc_.tile_pool(name="dirp", bufs=1) as dirp, \
             tc_.tile_pool(name="dsp", bufs=2) as sp, \
             tc_.tile_pool(name="dsp1", bufs=1) as sp1:
          sel_sb = dirp.tile([DTR, 2 * DS * 128], F16, tag="sel", name="sel")
          nc.sync.dma_start(sel_sb[:], dram["sel"][:])
          for di_ in range(2):
            p = ("m1_", "m2_")[di_]
            xnm = ("xTp", "xTrp")[di_]
            xc_scr, sz_scr = xc_scrs[di_], sz_scrs[di_]
            A_sb = dirp.tile([128, KI * DS], F32, tag=p + "A", name="A")
            nc.sync.dma_start(
                A_sb[:].rearrange("q (k s) -> q k s", k=KI),
                dram[p + "A"].rearrange("(k q) s -> q k s", q=128))
            vec = {}
            for nm in ("dt_b", "cb", "Dp"):
                vec[nm] = dirp.tile([128, KI], F32, tag=p + nm, name=nm)
                nc.sync.dma_start(vec[nm][:], dram[p + nm][:])
            xpw_sb = dirp.tile([128, KI * (DTR + 2 * DS)], F16, tag=p + "xpw",
                               name="xpw")
            nc.sync.dma_start(
                xpw_sb[:].rearrange("q (k c) -> q k c", k=KI),
                dram[p + "xproj_wT"].rearrange("(k q) c -> q k c", q=128))
            dtw_sb = dirp.tile([DTR, DI], MMDT, tag=p + "dtw", name="dtw")
            nc.sync.dma_start(dtw_sb[:], dram[p + "dt_wT"][:])
            carry = dirp.tile([128, KI * DS], F32, tag="carry", name="carry")
            nc.vector.memset(carry[:], 0.0)
            dbc_sb = dirp.tile([64, TC], MMDT, tag="dbc", name="dbc")
            dbc32 = dirp.tile([32, TC], F16, tag="dbc32", name="dbc32")
            a_sts = [dirp.tile([128, NST], SDT, tag=f"a_st{i}", name=f"a_st{i}")
                     for i in range(2)]
            b_stp = dirp.tile([128, NST], SDT, tag="b_stp", name="b_stp")
            nc.vector.memset(b_stp[:, 0::SEG], 0.0)
            for i_ in range(2):
                nc.vector.memset(
                    a_sts[i_][:].rearrange("q (s t) -> q s t", s=DS)[:, :, 0:2],
                    0.0)

            for tcix in range(NTC):
                t0 = tcix * TC
                # x window [t0-1, t0+TC) via the zero-padded xTp (col t of
                # xTp = x col t-1)
                xw = [sp.tile([128, TC + 1], MMDT, tag=f"xw{k}", name=f"xw{k}",
                              bufs=1) for k in range(KD)]
                for k in range(KD):
                    nc.sync.dma_start(
                        xw[k][:],
                        dram[xnm][k * 128:(k + 1) * 128, t0:t0 + TC + 1])

                # ---- P1 (Silu table): xc, dbc accumulation, silu(z) --------
                dbc_ps = psacc.tile([64, TC], F32, tag="acc_dbc", name="acc_dbc")
                for kt in range(KI):
                    ps = psmm.tile([128, TC], F32, tag="mm", name="mm")
                    w4 = wp.tile([128, 2 * KD * 128], MMDT, tag="w_in", name="w_in")
                    w01_r = dram[p + "W01T"].rearrange("(k q) e -> q k e", q=128)
                    nc.gpsimd.dma_start(
                        w4[:, 0:KD * 128].rearrange("q (k e) -> q k e", k=KD),
                        w01_r[:, :, kt * 128:(kt + 1) * 128])
                    nc.gpsimd.dma_start(
                        w4[:, KD * 128:].rearrange("q (k e) -> q k e", k=KD),
                        w01_r[:, :, DI + kt * 128:DI + (kt + 1) * 128])
                    for k in range(KD):            # W0 @ x_prev + W1 @ x
                        mm(ps[:], r32(w4[:, k * 128:(k + 1) * 128]),
                           xw[k][:, 0:TC], start=(k == 0), stop=False)
                    for k in range(KD):
                        mm(ps[:], r32(w4[:, (KD + k) * 128:(KD + k + 1) * 128]),
                           xw[k][:, 1:TC + 1], start=False, stop=(k == KD - 1))
                    xck = sp.tile([128, TC], F16, tag="xck", name="xck", bufs=1)
                    if sim_silu:
                        sg_ = sp.tile([128, TC], F32, tag="sg_", name="sg_", bufs=1)
                        vv_ = sp.tile([128, TC], F32, tag="vv_", name="vv_", bufs=1)
                        nc.scalar.activation(sg_[:], ps[:], AF.Sigmoid,
                                             bias=vec["cb"][:, kt:kt + 1])
                        nc.vector.tensor_scalar(vv_[:], ps[:],
                                                vec["cb"][:, kt:kt + 1],
                                                None, AL.add)
                        nc.vector.tensor_tensor(xck[:], sg_[:], vv_[:], AL.mult)
                    else:
                        nc.scalar.activation(xck[:], ps[:], AF.Silu,
                                             bias=vec["cb"][:, kt:kt + 1])
                    nc.gpsimd.dma_start(xc_scr[kt * 128:(kt + 1) * 128, t0:t0 + TC], xck[:])
                    mm(dbc_ps[:], xpw_sb[:, kt * 64:(kt + 1) * 64], xck[:],
                       start=(kt == 0), stop=(kt == KI - 1))

                    zps = psmm.tile([128, TC], F32, tag="mm", name="mm")
                    wz = wp.tile([128, KD * 128], MMDT, tag="w_z", name="w_z")
                    nc.gpsimd.dma_start(
                        wz[:].rearrange("q (k e) -> q k e", k=KD),
                        dram[p + "in_wzT"].rearrange("(k q) e -> q k e", q=128)
                        [:, :, kt * 128:(kt + 1) * 128])
                    for k in range(KD):
                        mm(zps[:], r32(wz[:, k * 128:(k + 1) * 128]),
                           xw[k][:, 1:TC + 1], start=(k == 0), stop=(k == KD - 1))
                    szk = sp.tile([128, TC], F16, tag="szk", name="szk", bufs=1)
                    if sim_silu:
                        sgz_ = sp.tile([128, TC], F32, tag="sgz_", name="sgz_", bufs=1)
                        nc.scalar.activation(sgz_[:], zps[:], AF.Sigmoid)
                        nc.vector.tensor_tensor(
                            szk[:], sgz_[:], zps[:], AL.mult)
                    else:
                        nc.scalar.activation(szk[:], zps[:], AF.Silu)
                    nc.gpsimd.dma_start(sz_scr[kt * 128:(kt + 1) * 128, t0:t0 + TC], szk[:])
                nc.scalar.copy(dbc_sb[:], dbc_ps[:])
                nc.scalar.copy(dbc32[:], dbc_sb[DTR:DTR + 2 * DS, :])

                # ---- broadcast B_s / C_s via selection matmuls -------------
                bcB = dirp.tile([128, DS * TC], SDT, tag=f"bcB{tcix % 2}",
                                name="bcB")
                bcC = dirp.tile([128, DS * TC], SDT, tag=f"bcC{tcix % 2}",
                                name="bcC")
                for s in range(2 * DS):
                    bps = psmm.tile([128, TC], F32, tag="mm", name="mm")
                    mm(bps[:], sel_sb[:, s * 128:(s + 1) * 128], dbc32[:],
                       start=True, stop=True)
                    dst = (bcB[:, s * TC:(s + 1) * TC] if s < DS
                           else bcC[:, (s - DS) * TC:(s - DS + 1) * TC])
                    nc.scalar.copy(dst, bps[:])

                # ---- P2 (Exp/Ln table): softplus dt, scan, y ---------------
                y_ps = [psacc.tile([128, TC], F32, tag=f"acc{k}", name=f"acc{k}")
                        for k in range(KD)]
                for kt in range(KI):
                    dps = psmm.tile([128, TC], F32, tag="mm", name="mm")
                    mm(dps[:], dtw_sb[:, kt * 128:(kt + 1) * 128],
                       dbc_sb[0:DTR, :], start=True, stop=True)
                    t1 = sp1.tile([128, TC], F32, tag="sp_t1", name="sp_t1")
                    nc.scalar.activation(t1[:], dps[:], AF.Exp,
                                         bias=vec["dt_b"][:, kt:kt + 1])
                    dts = sp.tile([128, TC], SDT, tag="dts", name="dts")
                    nc.scalar.activation(dts[:], t1[:], AF.Ln, bias=1.0)

                    # a-stream: per-state segments [0,0 | exp(A_s*dt)*TC]
                    a_st = a_sts[kt % 2]
                    for s in range(DS):
                        nc.scalar.activation(
                            a_st[:, s * SEG + 2:(s + 1) * SEG], dts[:], AF.Exp,
                            scale=A_sb[:, kt * DS + s:kt * DS + s + 1])

                    xck = sp.tile([128, TC], F16, tag="xck2", name="xck2")
                    nc.gpsimd.dma_start(xck[:], xc_scr[kt * 128:(kt + 1) * 128, t0:t0 + TC])
                    u = sp1.tile([128, TC], SDT, tag="u", name="u")
                    nc.vector.tensor_tensor(u[:], xck[:], dts[:], AL.mult)

                    # b-stream: [0,carry_s | u*B_s]
                    b_st = b_stp
                    nc.vector.tensor_copy(b_st[:, 1::SEG],
                                          carry[:, kt * DS:(kt + 1) * DS])
                    SG = 4
                    uv = u[:].rearrange("q (o t) -> q o t", o=1)
                    for sg in range(DS // SG):
                        nc.vector.tensor_tensor(
                            b_st[:].rearrange("q (s t) -> q s t", s=DS)
                            [:, sg * SG:(sg + 1) * SG, 2:SEG],
                            uv.to_broadcast((128, SG, TC)),
                            bcB[:, sg * SG * TC:(sg + 1) * SG * TC]
                            .rearrange("q (s t) -> q s t", s=SG), AL.mult)

                    h = sp1.tile([128, NST], SDT, tag="h", name="h")
                    nc.vector.tensor_tensor_scan(h[:], a_st[:], b_st[:], 0.0,
                                                 AL.mult, AL.add)
                    nc.vector.tensor_copy(carry[:, kt * DS:(kt + 1) * DS],
                                          h[:, TC + 1::SEG])

                    # y_t = sum_s C_s(t) h_s(t):  mult (2x fp16) + tree + reduce
                    hC = b_st
                    nc.vector.tensor_tensor(
                        hC[:, 0:DS * TC].rearrange("q (s t) -> q s t", s=DS),
                        h[:].rearrange("q (s t) -> q s t", s=DS)[:, :, 2:SEG],
                        bcC[:].rearrange("q (s t) -> q s t", s=DS), AL.mult)
                    nc.vector.tensor_tensor(hC[:, 0:DS * TC // 2],
                                            hC[:, 0:DS * TC // 2],
                                            hC[:, DS * TC // 2:DS * TC], AL.add)
                    nc.vector.tensor_tensor(hC[:, 0:DS * TC // 4],
                                            hC[:, 0:DS * TC // 4],
                                            hC[:, DS * TC // 4:DS * TC // 2],
                                            AL.add)
                    nc.vector.tensor_tensor(hC[:, 0:DS * TC // 8],
                                            hC[:, 0:DS * TC // 8],
                                            hC[:, DS * TC // 8:DS * TC // 4],
                                            AL.add)
                    yv = sp.tile([128, TC], SDT, tag="yv", name="yv")
                    nc.vector.tensor_tensor(yv[:], hC[:, 0:TC],
                                            hC[:, TC:2 * TC], AL.add)

                    # y += Dp*xc (Dp*xc on ACT: per-partition scale)
                    dpx = sp.tile([128, TC], SDT, tag="dpx", name="dpx")
                    nc.scalar.activation(dpx[:], xck[:], AF.Copy,
                                         scale=vec["Dp"][:, kt:kt + 1])
                    yf = sp.tile([128, TC], SDT, tag="yf", name="yf")
                    nc.vector.tensor_tensor(yf[:], yv[:], dpx[:], AL.add)
                    szk = sp.tile([128, TC], F16, tag="szk2", name="szk2")
                    nc.gpsimd.dma_start(szk[:], sz_scr[kt * 128:(kt + 1) * 128, t0:t0 + TC])
                    g = sp.tile([128, TC], F16, tag="g", name="g", bufs=2)
                    nc.vector.tensor_tensor(g[:], yf[:], szk[:], AL.mult)

                    w4 = wp.tile([128, KD * 128], F16, tag="w_out", name="w_out")
                    nc.gpsimd.dma_start(
                        w4[:], dram[p + "out_wT"][kt * 128:(kt + 1) * 128, :])
                    for k in range(KD):
                        mm(y_ps[k][:], w4[:, k * 128:(k + 1) * 128], g[:],
                           start=(kt == 0), stop=(kt == KI - 1))
                for k in range(KD):
                    yo = sp.tile([128, TC], F32, tag="yo", name="yo", bufs=1)
                    nc.scalar.copy(yo[:], y_ps[k][:])
                    nc.sync.dma_start(
                        y_scr[di_][k * 128:(k + 1) * 128, t0:t0 + TC], yo[:])

        # ============================================================ phase C
        with tc_.tile_pool(name="cpool", bufs=1) as cp, \
             tc_.tile_pool(name="csp", bufs=2) as sp:
            ln_g = cp.tile([128, KD], F32, tag="ln_g", name="ln_g")
            ln_b = cp.tile([128, KD], F32, tag="ln_b", name="ln_b")
            c1b = cp.tile([128, KF], F32, tag="c1b", name="c1b")
            c2b = cp.tile([128, KD], F32, tag="c2b", name="c2b")
            for nm, t in (("ln_g", ln_g), ("ln_b", ln_b), ("c1_b", c1b),
                          ("c2_b", c2b)):
                nc.sync.dma_start(t[:], dram[nm][:])
            c1w_sb = cp.tile([128, KD * DFF], F16, tag="c1w", name="c1w")
            nc.gpsimd.dma_start(
                c1w_sb[:].rearrange("q (k e) -> q k e", k=KD),
                dram["c1_wT"].rearrange("(k q) e -> q k e", q=128))
            c2w_sb = cp.tile([128, KF * D], F16, tag="c2w", name="c2w")
            nc.gpsimd.dma_start(
                c2w_sb[:].rearrange("q (f e) -> q f e", f=KF),
                dram["c2_wT"].rearrange("(f q) e -> q f e", q=128))
            CH = min(MMF, L)

            def layernorm(in_tiles, out_tiles):
                f16in = True
                one_st = ones16
                sums = cp.tile([1, L], MMDT, tag="ln_srow", name="ln_srow")
                sqs = cp.tile([1, L], MMDT, tag="ln_qrow", name="ln_qrow")
                for nk in range(0, L, CH):
                    sps = psacc.tile([1, CH], F32, tag="mmrow", name="mmrow")
                    for k in range(KD):
                        mm(sps[:], one_st[:, 0:1],
                           in_tiles[k][:, nk:nk + CH],
                           start=(k == 0), stop=(k == KD - 1))
                    nc.scalar.copy(sums[:, nk:nk + CH], sps[:])
                for nk in range(0, L, CH):
                    qps = psacc.tile([1, CH], F32, tag="mmrow", name="mmrow")
                    for k in range(KD):
                        sq = sp.tile([128, CH], F16, tag="ln_sq", name="ln_sq")
                        nc.vector.tensor_tensor(sq[:], in_tiles[k][:, nk:nk + CH],
                                                in_tiles[k][:, nk:nk + CH], AL.mult)
                        mm(qps[:], one_st[:, 0:1], sq[:],
                           start=(k == 0), stop=(k == KD - 1))
                    nc.scalar.copy(sqs[:, nk:nk + CH], qps[:])
                mu = cp.tile([128, L], F32, tag="ln_mu", name="ln_mu")
                inv = cp.tile([128, L], F32, tag="ln_inv", name="ln_inv")
                for nk in range(0, L, CH):
                    mps = psmm.tile([128, CH], F32, tag="mm", name="mm")
                    mm(mps[:], r32(ones_sb[0:1, :]), sums[:, nk:nk + CH],
                       start=True, stop=True)
                    nc.vector.tensor_scalar(mu[:, nk:nk + CH], mps[:], 1.0 / D,
                                            None, AL.mult)
                    qrep = psmm.tile([128, CH], F32, tag="mm", name="mm")
                    mm(qrep[:], r32(ones_sb[0:1, :]), sqs[:, nk:nk + CH],
                       start=True, stop=True)
                    ex2 = sp.tile([128, CH], F32, tag="ln_ex2", name="ln_ex2")
                    nc.vector.tensor_scalar(ex2[:], qrep[:], 1.0 / D, None, AL.mult)
                    var = sp.tile([128, CH], F32, tag="ln_var", name="ln_var")
                    nc.vector.tensor_tensor(var[:], mu[:, nk:nk + CH],
                                            mu[:, nk:nk + CH], AL.mult)
                    nc.vector.tensor_tensor(var[:], ex2[:], var[:], AL.subtract)
                    sd = sp.tile([128, CH], F32, tag="ln_sd", name="ln_sd")
                    nc.scalar.activation(sd[:], var[:], AF.Sqrt, bias=eps_sb[:])
                    nc.vector.reciprocal(inv[:, nk:nk + CH], sd[:])
                for k in range(KD):
                    xm = sp.tile([128, L], F32, tag="ln_xm", name="ln_xm")
                    nc.vector.tensor_tensor(xm[:], in_tiles[k][:], mu[:],
                                            AL.subtract)
                    nc.vector.tensor_tensor(xm[:], xm[:], inv[:], AL.mult)
                    nc.vector.tensor_scalar(out_tiles[k][:], xm[:],
                                            ln_g[:, k:k + 1], ln_b[:, k:k + 1],
                                            AL.mult, AL.add)

            y3p = [cp.tile([128, L], F16, tag=f"y3p{k}", name=f"y3p{k}")
                   for k in range(KD)]
            for k in range(KD):
                xt = sp.tile([128, L], MMDT, tag="c_x", name="c_x")
                y1t = sp.tile([128, L], F32, tag="c_y1", name="c_y1")
                y2t = sp.tile([128, L], F32, tag="c_y2", name="c_y2")
                nc.sync.dma_start(xt[:], dram["xTp"][k * 128:(k + 1) * 128, 1:L + 1])
                nc.sync.dma_start(y1t[:], y_scr[0][k * 128:(k + 1) * 128, :])
                nc.sync.dma_start(y2t[:], y_scr[1][k * 128:(k + 1) * 128, :])
                nc.vector.tensor_tensor(y3p[k][:], xt[:], y1t[:], AL.add)
                nc.vector.tensor_tensor(y3p[k][:], y3p[k][:], y2t[:, ::-1], AL.add)
            y3 = [cp.tile([128, L], F16, tag=f"y3_{k}", name=f"y3_{k}")
                  for k in range(KD)]
            layernorm(y3p, y3)

            ypre = y3p
            NFH = min(8, KF)
            for nk in range(0, L, CH):
                yacc = [psacc.tile([128, CH], F32, tag=f"acc{k}", name=f"acc{k}")
                        for k in range(KD)]
                for fh in range(KF // NFH):
                    hbuf = []
                    for f2 in range(NFH):
                        f = fh * NFH + f2
                        hps = psmm.tile([128, CH], F32, tag="mm", name="mm")
                        for k in range(KD):
                            mm(hps[:],
                               c1w_sb[:, k * DFF + f * 128:
                                      k * DFF + (f + 1) * 128],
                               y3[k][:, nk:nk + CH],
                               start=(k == 0), stop=(k == KD - 1))
                        hb = sp.tile([128, CH], F16, tag=f"hb{f2}",
                                     name=f"hb{f2}", bufs=1)
                        nc.scalar.activation(hb[:], hps[:], AF.Relu,
                                             bias=c1b[:, f:f + 1])
                        hbuf.append(hb)
                    for f2 in range(NFH):
                        f = fh * NFH + f2
                        for k in range(KD):
                            mm(yacc[k][:],
                               c2w_sb[:, f * D + k * 128:
                                      f * D + (k + 1) * 128],
                               hbuf[f2][:],
                               start=(f == 0), stop=(f == KF - 1))
                for k in range(KD):
                    nc.vector.scalar_tensor_tensor(
                        ypre[k][:, nk:nk + CH], yacc[k][:], c2b[:, k:k + 1],
                        y3[k][:, nk:nk + CH], AL.add, AL.add)
            outs = [cp.tile([128, L], F32, tag=f"yo_{k}", name=f"yo_{k}")
                    for k in range(KD)]
            layernorm(ypre, outs)
            for k in range(KD):
                nc.sync.dma_start(outT[k * 128:(k + 1) * 128, :],
                                  outs[k][:].bitcast(F32))


# ------------------------------------------------------------------ host side
_PROG_CACHE = {}


def _get_prog():
    if "full" not in _PROG_CACHE:
        _PROG_CACHE["full"] = build_program(DIMS)
    return _PROG_CACHE["full"]


def host_prep(inputs, dm: Dims = DIMS):
    f = np.float32
    x = np.asarray(inputs["x"], dtype=f)
    KI, KD, KF = dm.KI, dm.KD, dm.KF
    DI = dm.DI

    def vt(v, n):
        return np.ascontiguousarray(np.asarray(v, f).reshape(n, 128).T)

    c = {}
    sel = np.zeros((dm.DTR, 2 * dm.DS * 128), f)
    for s in range(2 * dm.DS):
        sel[s, s * 128:(s + 1) * 128] = 1.0
    c["sel"] = sel.astype(np.float16)
    c["ones"] = np.ones((128, 128), f)
    for p in ("m1_", "m2_"):
        in_w = np.asarray(inputs[p + "in_w"], f)          # (2DI, D)
        cw = np.asarray(inputs[p + "conv_w"], f)          # (DI, 2)
        W0 = cw[:, 0:1] * in_w[:DI]                       # (DI, D)
        W1 = cw[:, 1:2] * in_w[:DI]
        c[p + "W01T"] = np.ascontiguousarray(
            np.concatenate([W0, W1], axis=0).T)           # (D, 2DI)
        c[p + "in_wzT"] = np.ascontiguousarray(in_w[DI:].T)
        c[p + "xproj_wT"] = np.ascontiguousarray(
            np.asarray(inputs[p + "xproj_w"], np.float16).T)
        c[p + "dt_wT"] = np.ascontiguousarray(np.asarray(inputs[p + "dt_w"], f).T)
        c[p + "out_wT"] = np.ascontiguousarray(
            np.asarray(inputs[p + "out_w"], np.float16).T)
        c[p + "A"] = np.ascontiguousarray(-np.exp(np.asarray(inputs[p + "A_log"], f)))
        c[p + "dt_b"] = vt(inputs[p + "dt_b"], KI)
        c[p + "cb"] = vt(inputs[p + "conv_b"], KI)
        c[p + "Dp"] = vt(inputs[p + "Dp"], KI)
    c["ln_g"] = vt(inputs["ln_g"], KD)
    c["ln_b"] = vt(inputs["ln_b"], KD)
    c["c1_wT"] = np.ascontiguousarray(np.asarray(inputs["c1_w"], f).T)
    c["c1_b"] = vt(inputs["c1_b"], KF)
    c["c2_wT"] = np.ascontiguousarray(np.asarray(inputs["c2_w"], f).T)
    c["c2_b"] = vt(inputs["c2_b"], KD)

    in_maps = []
    for b in range(x.shape[0]):
        m = dict(c)
        xp = np.zeros((dm.D, dm.L + 1), f)
        xp[:, 1:] = x[b].T
        m["xTp"] = xp
        xrp = np.zeros((dm.D, dm.L + 1), f)
        xrp[:, 1:] = x[b][::-1].T
        m["xTrp"] = xrp
        in_maps.append(m)
    return in_maps


def kernel(**inputs):
    nc = _get_prog()
    in_maps = host_prep(inputs)
    res = bass_utils.run_bass_kernel_spmd(nc, in_maps, core_ids=list(range(NCORES)))
    return np.stack([np.ascontiguousarray(o["outT"].T) for o in res.results], axis=0)
